# revision 1
# baseline (speedup 1.0000x reference)
"""HD95 loss kernel for Trainium2 (Bass/Tile), 8 NeuronCores.

Reference semantics: per image, threshold pred/true at 0.5, compact nonzero
pixel indices in row-major order, split each point list into blocks of 1000,
and for every (point, opposite-side block) pair take the min Euclidean
distance; the HD95 is the 95th linear-interpolation quantile over all finite
such mins (both directions), averaged over the batch.

Device algorithm (per image & direction, "queries" vs "ref blocks"):
separable squared-EDT. All coordinates are integers < 96, and every operand
is decomposed into bf16-exact integer parts (squares split into a multiple
of 128 plus a <128 remainder), so every matmul product is exact in the fp32
PSUM accumulator and the result is bit-exact vs the reference.

  stage 1:  g[x, c] = min_{a : pixel(b0+c, a) in block} (x-a)^2
            contraction-5 bf16 matmul ([x2h,x2l,x,1,1] x [1,1,-2a,a2h,a2l],
            sentinel columns [0,0,0,2^26,0]) over a <=24-row candidate
            window per block, then a DVE min-reduce per 96-col group.
  split:    g -> g_hi (multiple of 128) + g_lo (<128), both bf16-exact.
  stage 2:  min d^2(q, blk) = min_c ( (y_q - (b0+c))^2 + g[x_q, c] )
            three accumulating bf16 matmuls per 128-query tile:
            onehot(x_q) @ g_hi, onehot(x_q) @ g_lo (the g gather), and
            [y2h,y2l,y,1,1] @ [1,1,-2b,b2h,b2l] (the (y-b)^2 term);
            then a DVE min-reduce over the 24 candidates of each block.

Core mapping: 8 cores = 4 (image x direction) jobs x 2 halves of 2560
query slots. Host does the O(N) compaction/feature build and the final
O(50k) quantile; device does all O(K x window) distance work.
"""

import numpy as np

H = 96
W = 96
BLK = 1000        # reference cdist block size
NBLK = 5          # blocks per side (asserted from the data regime)
CAND = 24         # candidate image rows per block window (spans <= 23 here)
CHUNK = 384       # stage-1 matmul free size (4 candidate rows)
CPG = 3           # stage-1 matmul chunks per group (psum tile = 3 banks)
NG1 = NBLK * CAND * 96 // (CHUNK * CPG)  # 10 stage-1 groups, 12 cands each
QHALF = 2560      # query slots per core (20 tiles of 128)
NTILES = QHALF // 128
BIG = float(2 ** 26)  # sentinel (bf16-exact, >> max real d^2 of 18050)
NCORES = 8

_CACHE = {}


def _build_nc():
    import concourse.bacc as bacc
    import concourse.mybir as mybir
    import concourse.tile as tile

    f32 = mybir.dt.float32
    bf16 = mybir.dt.bfloat16
    # Bacc (not raw Bass): its compile() runs move_matmul_waits_to_ldweights
    # + generate_event_semaphores, which legalize multi-wait instructions
    # (TRN2 allows at most one sync wait per instruction).
    nc = bacc.Bacc("TRN2", target_bir_lowering=False, debug=False)

    s1_pack = nc.declare_dram_parameter(
        "s1_pack", [5, 96 + NBLK * CAND * 96], bf16, isOutput=False
    )
    s2_lhsT = nc.declare_dram_parameter(
        "s2_lhsT", [101, NTILES * 128], bf16, isOutput=False
    )
    s2_rtop = nc.declare_dram_parameter(
        "s2_rtop", [5, NBLK * CAND], bf16, isOutput=False
    )
    mins = nc.declare_dram_parameter(
        "mins", [128, NTILES * NBLK], f32, isOutput=True
    )

    X = mybir.AxisListType.X
    MIN = mybir.AluOpType.min

    with tile.TileContext(nc) as tc:
        with (
            tc.tile_pool(name="const", bufs=1) as const,
            tc.tile_pool(name="ps1", bufs=2, space="PSUM") as ps1,
            tc.tile_pool(name="ps2", bufs=2, space="PSUM") as ps2,
        ):
            t_s1 = const.tile([5, 96 + NBLK * CAND * 96], bf16)
            t_s2_lhsT = const.tile([101, NTILES * 128], bf16)
            t_rhs2 = const.tile([96, NBLK * CAND], f32)
            t_tmp32 = const.tile([96, NBLK * CAND], f32)
            t_gh32 = const.tile([96, NBLK * CAND], f32)
            # rows 0..95: g_hi (ACT-written); rows 96..100: rtop (DMA)
            t_ghr = const.tile([101, NBLK * CAND], bf16)
            t_gl = const.tile([96, NBLK * CAND], bf16)
            t_out = const.tile([128, NTILES * NBLK], f32)
            t_s1_lhsT = t_s1[:, 0:96]

            # split the critical-path DMA across 4 HWDGE queues
            n1 = 96 + NBLK * CAND * 96
            for i in range(4):
                sl = slice(i * n1 // 4, (i + 1) * n1 // 4)
                nc.sync.dma_start(t_s1[:, sl], s1_pack[:, sl])
            nc.sync.dma_start(t_s2_lhsT[:], s2_lhsT[:])
            nc.sync.dma_start(t_ghr[96:101, :], s2_rtop[:])

            # stage 1: g[x, (blk, cand)] -> t_rhs2, 12 candidate rows/group
            for gi in range(NG1):
                ps = ps1.tile([96, CPG, 512], f32, tag="ps1")
                for k in range(CPG):
                    c0 = 96 + (gi * CPG + k) * CHUNK
                    nc.tensor.matmul(
                        ps[:, k, 0:CHUNK],
                        t_s1_lhsT,
                        t_s1[:, c0 : c0 + CHUNK],
                        start=True,
                        stop=True,
                    )
                # [96, 3, 384] -> [96, 3, 4, 96], min over innermost
                red_in = ps[:, :, 0:CHUNK].rearrange("p c (u a) -> p c u a", a=96)
                o0 = gi * (CAND // 2)
                nc.vector.tensor_reduce(
                    t_rhs2[0:96, o0 : o0 + CAND // 2], red_in, axis=X, op=MIN
                )

            # split g into bf16-exact hi/lo parts for the stage-2 gather:
            # hi = round(g/128)*128 via the +2^23 float-rounding trick,
            # lo = g - hi in [-64, 64) -- both exact in bf16, sum exact.
            # The rounding runs on the idle Scalar engine (out = Copy(
            # in*scale + bias)); only the subtract needs the Vector engine.
            P23 = float(2 ** 23)
            COPY = mybir.ActivationFunctionType.Copy
            nc.scalar.activation(
                t_tmp32[:], t_rhs2[:], COPY, bias=P23, scale=1.0 / 128.0
            )
            nc.scalar.activation(
                t_gh32[:], t_tmp32[:], COPY, bias=-P23 * 128.0, scale=128.0
            )
            nc.scalar.activation(t_ghr[0:96, :], t_gh32[:], COPY)
            nc.vector.tensor_sub(t_gl[:], t_rhs2[:], t_gh32[:])

            # stage 2: two tiles per PSUM bank; per tile two accumulating
            # matmuls ([onehot;yfeat] @ [g_hi;rtop], then onehot @ g_lo),
            # then one paired min-reduce
            for t2 in range(NTILES // 2):
                ps_o = ps2.tile([128, 2, NBLK, CAND], f32, tag="ps2")
                for h in range(2):
                    t = 2 * t2 + h
                    ts = slice(t * 128, (t + 1) * 128)
                    nc.tensor.matmul(
                        ps_o[:, h, :, :], t_s2_lhsT[:, ts], t_ghr[:],
                        start=True, stop=False,
                    )
                    nc.tensor.matmul(
                        ps_o[:, h, :, :], t_s2_lhsT[0:96, ts], t_gl[:],
                        start=False, stop=True,
                    )
                nc.vector.tensor_reduce(
                    t_out[:, t2 * 2 * NBLK : (t2 + 1) * 2 * NBLK],
                    ps_o[:, :, :, :], axis=X, op=MIN,
                )

            nc.sync.dma_start(mins[:], t_out[:])

    nc.compile()
    return nc


def _get_nc():
    if "nc" not in _CACHE:
        _CACHE["nc"] = _build_nc()
    return _CACHE["nc"]


def _bf16(a):
    from ml_dtypes import bfloat16

    return np.asarray(a, np.float32).astype(bfloat16)


def _hilo(v):
    """Split integer-valued array into (multiple-of-128, remainder<128)."""
    v = np.asarray(v, np.float64)
    lo = np.mod(v, 128.0)
    return (v - lo).astype(np.float32), lo.astype(np.float32)


def _side_points(img):
    """Compacted nonzero pixel coords, row-major ascending (matches
    jnp.nonzero order)."""
    m = (np.asarray(img) > 0.5).reshape(-1)
    idx = np.nonzero(m)[0]
    ys = (idx // W).astype(np.int64)
    xs = (idx % W).astype(np.int64)
    return ys, xs


def _feat5_queries(vals):
    """[v2h, v2l, v, 1, 1] feature rows for the squared-term side."""
    v = np.asarray(vals, np.float64)
    h, l = _hilo(v * v)
    one = np.ones_like(v, np.float32)
    return np.stack([h, l, v.astype(np.float32), one, one])


def _feat5_refs(vals):
    """[1, 1, -2v, v2h, v2l] feature rows for the reference side."""
    v = np.asarray(vals, np.float64)
    h, l = _hilo(v * v)
    one = np.ones_like(v, np.float32)
    return np.stack([one, one, (-2.0 * v).astype(np.float32), h, l])


def _build_core_inputs(q_ys, q_xs, r_ys, r_xs):
    """Host-side feature build for one (image, direction) job.

    q_*: query points (cnt_q), r_*: reference points (cnt_r, split into
    NBLK blocks of BLK in compacted order). Returns two per-core input
    maps, or None if the data falls outside the compiled regime.
    """
    cnt_q, cnt_r = len(q_ys), len(r_ys)
    if not (0 < cnt_q <= 2 * QHALF and 0 < cnt_r <= NBLK * BLK):
        return None
    if (cnt_r + BLK - 1) // BLK != NBLK:
        return None

    s1_lhsT = _feat5_queries(np.arange(96))

    s1_rhs = np.zeros((5, NBLK, CAND, 96), np.float32)
    s1_rhs[3] = BIG  # sentinel [0, 0, 0, BIG, 0]
    s2_rtop = np.empty((5, NBLK, CAND), np.float32)
    for blk in range(NBLK):
        lo, hi = blk * BLK, min((blk + 1) * BLK, cnt_r)
        ys_b, xs_b = r_ys[lo:hi], r_xs[lo:hi]
        b0 = int(ys_b[0])
        if int(ys_b[-1]) - b0 + 1 > CAND:
            return None
        s1_rhs[:, blk, ys_b - b0, xs_b] = _feat5_refs(xs_b)
        s2_rtop[:, blk, :] = _feat5_refs(b0 + np.arange(CAND))
    s1_pack = _bf16(np.concatenate([s1_lhsT, s1_rhs.reshape(5, -1)], axis=1))

    # stage-2 lhsT rows: 0..95 onehot(x), 96..100 yfeat; padded slots zero
    s2_lhsT = np.zeros((101, 2 * QHALF), np.float32)
    s2_lhsT[q_xs, np.arange(cnt_q)] = 1.0
    s2_lhsT[96:101, :cnt_q] = _feat5_queries(q_ys)

    maps = []
    for half in range(2):
        hs = slice(half * QHALF, (half + 1) * QHALF)
        maps.append(
            {
                "s1_pack": s1_pack,
                "s2_lhsT": _bf16(s2_lhsT[:, hs]),
                "s2_rtop": _bf16(s2_rtop.reshape(5, -1)),
            }
        )
    return maps


def _quantile95(vals):
    """torch.quantile / jnp.nanquantile 'linear' on finite values."""
    v = np.sort(np.asarray(vals, np.float64))
    n = v.size
    pos = 0.95 * (n - 1)
    lo = int(np.floor(pos))
    hi = min(lo + 1, n - 1)
    frac = pos - lo
    return v[lo] * (1.0 - frac) + v[hi] * frac


def _hd95_numpy_fallback(pred, true):
    """Pure-numpy path for data outside the compiled regime."""
    p_ys, p_xs = _side_points(pred)
    t_ys, t_xs = _side_points(true)
    if len(p_ys) == 0 or len(t_ys) == 0:
        return None
    pc = np.stack([p_ys, p_xs], -1).astype(np.float32)
    tc = np.stack([t_ys, t_xs], -1).astype(np.float32)
    vals = []
    for qc, rc in ((pc, tc), (tc, pc)):
        nbr = (len(rc) + BLK - 1) // BLK
        for jb in range(nbr):
            b = rc[jb * BLK : (jb + 1) * BLK]
            d2 = (
                (qc * qc).sum(-1)[:, None]
                + (b * b).sum(-1)[None, :]
                - 2.0 * (qc @ b.T)
            )
            vals.append(np.sqrt(np.maximum(d2.min(1), 0.0).astype(np.float32)))
    return _quantile95(np.concatenate(vals))


def _run_device(in_maps, trace=False):
    from concourse.bass_utils import run_bass_kernel_spmd

    nc = _get_nc()
    return run_bass_kernel_spmd(nc, in_maps, list(range(NCORES)), trace=trace)


def kernel(input, target, _trace=False, _results_out=None):
    input = np.asarray(input)
    target = np.asarray(target)
    nimg = input.shape[0]

    # jobs: (image, direction). dir 0: queries=pred, refs=true (row mins);
    # dir 1: queries=true, refs=pred (col mins).
    jobs = []
    in_maps = []
    fallback = {}
    ok_mask = []
    for i in range(nimg):
        p_ys, p_xs = _side_points(input[i])
        t_ys, t_xs = _side_points(target[i])
        ok = len(p_ys) > 0 and len(t_ys) > 0
        ok_mask.append(ok)
        if not ok:
            continue
        built_row = _build_core_inputs(p_ys, p_xs, t_ys, t_xs)
        built_col = _build_core_inputs(t_ys, t_xs, p_ys, p_xs)
        if built_row is None or built_col is None or nimg != 2:
            fallback[i] = _hd95_numpy_fallback(input[i], target[i])
            continue
        jobs.append((i, 0, len(p_ys)))
        in_maps.extend(built_row)
        jobs.append((i, 1, len(t_ys)))
        in_maps.extend(built_col)

    hds = {}
    if jobs:
        while len(in_maps) < NCORES:  # pad to the full 8-core SPMD launch
            in_maps.append({k: v.copy() for k, v in in_maps[0].items()})
        res = _run_device(in_maps[:NCORES], trace=_trace)
        if _results_out is not None:
            _results_out.append(res)
        per_img_vals = {}
        for j, (img, _dir, cnt_q) in enumerate(jobs):
            o0 = res.results[2 * j]["mins"]      # [128, NTILES*NBLK]
            o1 = res.results[2 * j + 1]["mins"]
            d2 = np.concatenate(
                [
                    o0.reshape(128, NTILES, NBLK).transpose(1, 0, 2),
                    o1.reshape(128, NTILES, NBLK).transpose(1, 0, 2),
                ]
            ).reshape(2 * QHALF, NBLK)[:cnt_q]
            assert d2.max() < 2.0 ** 25, "sentinel leaked into mins"
            dist = np.sqrt(d2.astype(np.float32))
            per_img_vals.setdefault(img, []).append(dist.ravel())
        for img, chunks in per_img_vals.items():
            hds[img] = _quantile95(np.concatenate(chunks))
    hds.update(fallback)

    n_ok = sum(ok_mask)
    if n_ok == 0:
        return np.float32(np.inf)
    total = sum(hds[i] for i in range(nimg) if ok_mask[i])
    return np.float32(total / n_ok)



# revision 3
# speedup vs baseline: 2.2524x; 2.2524x over previous
"""HD95 loss kernel for Trainium2 (Bass/Tile), 8 NeuronCores.

Reference semantics: per image, threshold pred/true at 0.5, compact nonzero
pixel indices in row-major order, split each point list into blocks of 1000,
and for every (point, opposite-side block) pair take the min Euclidean
distance; the HD95 is the 95th linear-interpolation quantile over all finite
such mins (both directions), averaged over the batch.

Device algorithm (per image & direction): grid-EDT. Every query is a pixel
of the 96x96 grid, so the device computes, for ALL grid pixels (y, x) and
every ref block, min_c [ (y - t_c)^2 + g[x, blk, c] ] where t_c = b0+c runs
over the block's <=23 candidate image rows and g[x, blk, c] is the 1-D
row-EDT min_a (x - a)^2 over the block's points in row t_c (host-prepared,
O(rows x 96) two-pointer work). Each candidate is evaluated exactly with one
contraction-5 bf16 matmul column [1, 1, -2t, Bhi, Blo] against stationary
y-features [y2h, y2l, y, 1, 1] (squares split into a multiple-of-128 part
plus a <128 remainder, so every product is bf16-exact and the fp32 PSUM sum
is the exact integer d^2). DVE min-reduces over candidates give
d2[y, (x, blk)]; the host gathers the actual query pixels, takes sqrt, and
computes the final quantile.

Core mapping: 8 cores = 4 (image x direction) jobs x 2 x-halves of the
grid. Per core: 12 matmuls (460 cols), 3 min-reduces, 3 input + 3 output
DMAs. Host does O(N) compaction, the tiny row-EDT table, and the O(50k)
quantile; device does all O(grid x window) distance evaluation and minima.
"""

import numpy as np

H = 96
W = 96
BLK = 1000        # reference cdist block size
NBLK = 5          # blocks per side (asserted from the data regime)
CAND = 23         # candidate image rows per block window (max actual span)
XH = 48           # x columns per core (half the grid)
CXM = 4           # x values per matmul chunk
CPB = NBLK * CAND           # 115 candidate cols per x value
MMF = CXM * CPB             # 460 matmul free size (<= 512 PSUM bank)
NMM = XH // CXM             # 12 matmuls per core
BPT = 4                     # matmul chunks (PSUM banks) per tile
NT = NMM // BPT             # 3 PSUM tiles -> 3 reduces, 3 output DMAs
NCOL = XH * CPB             # 5520 rhs cols per core
BIG = float(2 ** 26)  # sentinel (bf16-exact, >> max real d^2 of ~20k)
NCORES = 8

_CACHE = {}


def _build_nc():
    import concourse.bacc as bacc
    import concourse.mybir as mybir
    import concourse.tile as tile

    f32 = mybir.dt.float32
    bf16 = mybir.dt.bfloat16
    # Bacc (not raw Bass): its compile() runs move_matmul_waits_to_ldweights
    # + generate_event_semaphores, which legalize multi-wait instructions
    # (TRN2 allows at most one sync wait per instruction).
    nc = bacc.Bacc("TRN2", target_bir_lowering=False, debug=False)

    pack = nc.declare_dram_parameter("pack", [5, 96 + NCOL], bf16, isOutput=False)
    mins = nc.declare_dram_parameter("mins", [96, NT * BPT * CXM * NBLK], f32,
                                     isOutput=True)

    X = mybir.AxisListType.X
    MIN = mybir.AluOpType.min

    with tile.TileContext(nc) as tc:
        with (
            tc.tile_pool(name="const", bufs=1) as const,
            tc.tile_pool(name="ps", bufs=2, space="PSUM") as psp,
        ):
            t_in = const.tile([5, 96 + NCOL], bf16)
            t_out = const.tile([96, NT * BPT * CXM * NBLK], f32)
            t_lhsT = t_in[:, 0:96]

            # input DMA split by PSUM-tile so matmuls start on the first
            # piece while the rest streams (separate HWDGE queues)
            piece = BPT * MMF  # 1840 cols per tile
            nc.sync.dma_start(t_in[:, 0 : 96 + piece], pack[:, 0 : 96 + piece])
            for t in range(1, NT):
                sl = slice(96 + t * piece, 96 + (t + 1) * piece)
                nc.sync.dma_start(t_in[:, sl], pack[:, sl])

            for t in range(NT):
                ps = psp.tile([96, BPT, 512], f32, tag="ps")
                for k in range(BPT):
                    c0 = 96 + (t * BPT + k) * MMF
                    nc.tensor.matmul(
                        ps[:, k, 0:MMF],
                        t_lhsT,
                        t_in[:, c0 : c0 + MMF],
                        start=True,
                        stop=True,
                    )
                # [96, BPT, (CXM, NBLK, CAND)] -> min over candidates
                red_in = ps[:, :, 0:MMF].rearrange(
                    "p b (x j c) -> p b (x j) c", x=CXM, j=NBLK, c=CAND
                )
                o0 = t * BPT * CXM * NBLK
                o1 = (t + 1) * BPT * CXM * NBLK
                nc.vector.tensor_reduce(
                    t_out[:, o0:o1], red_in, axis=X, op=MIN
                )
                nc.sync.dma_start(mins[:, o0:o1], t_out[:, o0:o1])

    nc.compile()
    return nc


def _get_nc():
    if "nc" not in _CACHE:
        _CACHE["nc"] = _build_nc()
    return _CACHE["nc"]


def _bf16(a):
    from ml_dtypes import bfloat16

    return np.asarray(a, np.float32).astype(bfloat16)


def _hilo(v):
    """Split integer-valued array into (multiple-of-128, remainder<128)."""
    v = np.asarray(v, np.float64)
    lo = np.mod(v, 128.0)
    return (v - lo).astype(np.float32), lo.astype(np.float32)


def _side_points(img):
    """Compacted nonzero pixel coords, row-major ascending (matches
    jnp.nonzero order)."""
    m = (np.asarray(img) > 0.5).reshape(-1)
    idx = np.nonzero(m)[0]
    ys = (idx // W).astype(np.int64)
    xs = (idx % W).astype(np.int64)
    return ys, xs


def _g_table(r_ys, r_xs):
    """Host row-EDT: B[x, blk, c] = t^2 + min_a (x-a)^2 over block blk's
    points in image row t = b0(blk)+c, or the BIG sentinel for empty
    candidate rows. Returns None if the data falls outside the compiled
    regime (not 5 blocks, or a block row-span > CAND)."""
    cnt = len(r_ys)
    if not (4 * BLK < cnt <= NBLK * BLK):
        return None
    B = np.full((96, NBLK, CAND), BIG, np.float64)
    xg = np.arange(96)
    for j in range(NBLK):
        lo, hi = j * BLK, min((j + 1) * BLK, cnt)
        ys_b, xs_b = r_ys[lo:hi], r_xs[lo:hi]
        b0 = int(ys_b[0])
        if int(ys_b[-1]) - b0 + 1 > CAND:
            return None
        # per-candidate-row slices of the (row-major sorted) point list
        starts = np.searchsorted(ys_b, b0 + np.arange(CAND), side="left")
        ends = np.searchsorted(ys_b, b0 + np.arange(CAND), side="right")
        for c in range(CAND):
            s, e = starts[c], ends[c]
            if s == e:
                continue  # empty candidate row -> sentinel
            a = xs_b[s:e]  # ascending x's present in this row-block
            i = np.searchsorted(a, xg).clip(1, e - s - 1) if e - s > 1 else \
                np.zeros(96, np.int64)
            if e - s > 1:
                d = np.minimum(np.abs(xg - a[i - 1]), np.abs(a[i] - xg))
            else:
                d = np.abs(xg - a[0])
            t = float(b0 + c)
            B[:, j, c] = t * t + d.astype(np.float64) ** 2
    return B


def _build_job_packs(r_ys, r_xs):
    """Packed [5, 96+NCOL] bf16 inputs for the two cores of one job."""
    Bt = _g_table(r_ys, r_xs)
    if Bt is None:
        return None
    y = np.arange(96, dtype=np.float64)
    y2h, y2l = _hilo(y * y)
    one = np.ones(96, np.float32)
    lhsT = np.stack([y2h, y2l, y.astype(np.float32), one, one])  # [5, 96]

    b0s = np.array([int(r_ys[j * BLK]) for j in range(NBLK)], np.float64)
    t = b0s[:, None] + np.arange(CAND)[None, :]        # [NBLK, CAND]
    tneg2 = np.broadcast_to(-2.0 * t, (96, NBLK, CAND))
    Bhi, Blo = _hilo(Bt)
    ones = np.ones((96, NBLK, CAND), np.float32)
    # rhs rows [1, 1, -2t, Bhi, Blo] per column (x, blk, c)
    rhs = np.stack([ones, ones, tneg2.astype(np.float32), Bhi, Blo])
    rhs = rhs.reshape(5, 96, CPB)

    packs = []
    for h in range(2):
        cols = rhs[:, h * XH : (h + 1) * XH, :].reshape(5, NCOL)
        packs.append({"pack": _bf16(np.concatenate([lhsT, cols], axis=1))})
    return packs


def _quantile95(vals):
    """torch.quantile / jnp.nanquantile 'linear' on finite values."""
    v = np.sort(np.asarray(vals, np.float64))
    n = v.size
    pos = 0.95 * (n - 1)
    lo = int(np.floor(pos))
    hi = min(lo + 1, n - 1)
    frac = pos - lo
    return v[lo] * (1.0 - frac) + v[hi] * frac


def _hd95_numpy_fallback(pred, true):
    """Pure-numpy path for data outside the compiled regime."""
    p_ys, p_xs = _side_points(pred)
    t_ys, t_xs = _side_points(true)
    if len(p_ys) == 0 or len(t_ys) == 0:
        return None
    pc = np.stack([p_ys, p_xs], -1).astype(np.float32)
    tc = np.stack([t_ys, t_xs], -1).astype(np.float32)
    vals = []
    for qc, rc in ((pc, tc), (tc, pc)):
        nbr = (len(rc) + BLK - 1) // BLK
        for jb in range(nbr):
            b = rc[jb * BLK : (jb + 1) * BLK]
            d2 = (
                (qc * qc).sum(-1)[:, None]
                + (b * b).sum(-1)[None, :]
                - 2.0 * (qc @ b.T)
            )
            vals.append(np.sqrt(np.maximum(d2.min(1), 0.0).astype(np.float32)))
    return _quantile95(np.concatenate(vals))


def _run_device(in_maps, trace=False):
    from concourse.bass_utils import run_bass_kernel_spmd

    nc = _get_nc()
    return run_bass_kernel_spmd(nc, in_maps, list(range(NCORES)), trace=trace)


def kernel(input, target, _trace=False, _results_out=None):
    input = np.asarray(input)
    target = np.asarray(target)
    nimg = input.shape[0]

    # jobs: (image, direction). dir 0: queries=pred, refs=true (row mins);
    # dir 1: queries=true, refs=pred (col mins). 2 cores per job (x halves).
    jobs = []
    in_maps = []
    fallback = {}
    ok_mask = []
    pts = {}
    for i in range(nimg):
        pts[i, 0] = _side_points(input[i])
        pts[i, 1] = _side_points(target[i])
        ok = len(pts[i, 0][0]) > 0 and len(pts[i, 1][0]) > 0
        ok_mask.append(ok)
        if not ok:
            continue
        built_row = _build_job_packs(*pts[i, 1])  # refs = true
        built_col = _build_job_packs(*pts[i, 0])  # refs = pred
        if built_row is None or built_col is None or nimg != 2:
            fallback[i] = _hd95_numpy_fallback(input[i], target[i])
            continue
        jobs.append((i, 0))
        in_maps.extend(built_row)
        jobs.append((i, 1))
        in_maps.extend(built_col)

    hds = {}
    if jobs:
        while len(in_maps) < NCORES:  # pad to the full 8-core SPMD launch
            in_maps.append({k: v.copy() for k, v in in_maps[0].items()})
        res = _run_device(in_maps[:NCORES], trace=_trace)
        if _results_out is not None:
            _results_out.append(res)
        per_img_vals = {}
        for j, (img, dr) in enumerate(jobs):
            o0 = res.results[2 * j]["mins"]      # [96, XH*NBLK] x in [0,48)
            o1 = res.results[2 * j + 1]["mins"]  # x in [48,96)
            d2 = np.concatenate(
                [o0.reshape(96, XH, NBLK), o1.reshape(96, XH, NBLK)], axis=1
            )  # [y, x, blk]
            q_ys, q_xs = pts[img, dr]
            qv = d2[q_ys, q_xs, :]
            assert qv.max() < 2.0 ** 25, "sentinel leaked into mins"
            per_img_vals.setdefault(img, []).append(
                np.sqrt(qv.astype(np.float32)).ravel()
            )
        for img, chunks in per_img_vals.items():
            hds[img] = _quantile95(np.concatenate(chunks))
    hds.update(fallback)

    n_ok = sum(ok_mask)
    if n_ok == 0:
        return np.float32(np.inf)
    total = sum(hds[i] for i in range(nimg) if ok_mask[i])
    return np.float32(total / n_ok)
